# revision 5
# baseline (speedup 1.0000x reference)
"""Extended Kalman Filter kernel for 8 Trainium2 NeuronCores.

Math: the EKF covariance recursion (P -> A P A^T + Q; S = C P C^T + R;
K = P C^T S^-1; P -> (I-KC)P) does not depend on the data, only on cov0.
When cov0 is identical across the batch (it is: broadcast 0.1*I), the
per-timestep Kalman gains K_t are batch-independent and can be
precomputed on the host. The device-side work collapses to a linear
time-varying recursion on the mean only:

    mean_{t+1} = M_t @ mean_t + N_t @ u_t + K_t @ z_t
    M_t = (I - K_t C) A,  N_t = (I - K_t C) Bm

Device mapping (pure data-parallel over batch, 4096 batch/core):
  * batch n in [0,4096) is split as n = h*2048 + p*16 + q with h in {0,1},
    p in [0,128) (SBUF partition of the batch-major staging tiles),
    q in [0,16) (position within a partition's contiguous 16-batch run).
  * On-chip state layout is "feature-major blocks": mean tile
    [96 = (q,i), 256 = (h,p)] so the recursion is a matmul with a
    block-diagonal stationary kron(I_16, M_t^T) of shape [96, 96].
  * u_t / z_t arrive batch-major (contiguous DMA), are transposed
    on the TensorEngine ([128, 96] / [128, 48] tiles -> PSUM), copied to
    SBUF, and injected with block-diagonal stationaries.
  * The updated mean (= the output for step t) is transposed back to
    batch-major on the TensorEngine and stored contiguously.
"""

import numpy as np

T, BFULL, D, O, U = 64, 32768, 6, 3, 6
NCORES = 8
BS = BFULL // NCORES      # 4096 batch per core
G = 16                    # batches per 6-row feature block (96 = G*D rows)
COLS = 256                # state columns = 2 halves * 128 partitions
KT = 8                    # timesteps per DMA staging group

_CACHE = {}
LAST_RESULTS = None       # BassKernelResults of the most recent device run


def _host_coeffs(cov0_row, A, Bm, Q_tril, C, R_tril):
    """Run the (batch-independent) covariance recursion on the host in
    float64; return per-step float32 coefficient matrices M_t, N_t, K_t."""
    A = np.asarray(A, np.float64)
    Bm = np.asarray(Bm, np.float64)
    Qt = np.asarray(Q_tril, np.float64)
    C = np.asarray(C, np.float64)
    Rt = np.asarray(R_tril, np.float64)
    Qc = Qt @ Qt.T
    Rc = Rt @ Rt.T
    P = np.asarray(cov0_row, np.float64)
    I = np.eye(D)
    Ms = np.empty((T, D, D), np.float32)
    Ns = np.empty((T, D, U), np.float32)
    Ks = np.empty((T, D, O), np.float32)
    for t in range(T):
        Pp = A @ P @ A.T + Qc
        S = C @ Pp @ C.T + Rc
        K = Pp @ C.T @ np.linalg.inv(S)
        IKC = I - K @ C
        Ms[t] = IKC @ A
        Ns[t] = IKC @ Bm
        Ks[t] = K
        P = IKC @ Pp
    return Ms, Ns, Ks


def _stationaries(Ms, Ns, Ks):
    """Block-diagonal lhsT stationaries. matmul computes lhsT.T @ rhs, so
    lhsT[(g,j),(g,i)] = M[i,j] i.e. each diagonal block is M^T."""
    SM = np.zeros((T, G * D, G * D), np.float32)
    SN = np.zeros((T, G * U, G * D), np.float32)
    SK = np.zeros((T, G * O, G * D), np.float32)
    for g in range(G):
        SM[:, g * D:(g + 1) * D, g * D:(g + 1) * D] = np.transpose(Ms, (0, 2, 1))
        SN[:, g * U:(g + 1) * U, g * D:(g + 1) * D] = np.transpose(Ns, (0, 2, 1))
        SK[:, g * O:(g + 1) * O, g * D:(g + 1) * D] = np.transpose(Ks, (0, 2, 1))
    return SM, SN, SK


def _build_program():
    """Build (once) the Bass/Tile program shared by all 8 cores."""
    if "nc" in _CACHE:
        return _CACHE["nc"]

    import concourse.bacc as bacc
    import concourse.tile as tile
    from concourse import mybir

    f32 = mybir.dt.float32
    nc = bacc.Bacc("TRN2", target_bir_lowering=False, debug=False,
                   num_devices=NCORES)

    meas = nc.dram_tensor("meas", [T, BS, O], f32, kind="ExternalInput").ap()
    useq = nc.dram_tensor("useq", [T, BS, U], f32, kind="ExternalInput").ap()
    mean0 = nc.dram_tensor("mean0", [BS, D], f32, kind="ExternalInput").ap()
    statM = nc.dram_tensor("statM", [T, G * D, G * D], f32, kind="ExternalInput").ap()
    statN = nc.dram_tensor("statN", [T, G * U, G * D], f32, kind="ExternalInput").ap()
    statK = nc.dram_tensor("statK", [T, G * O, G * D], f32, kind="ExternalInput").ap()
    ident = nc.dram_tensor("ident", [128, 128], f32, kind="ExternalInput").ap()
    out = nc.dram_tensor("out", [T, BS, D], f32, kind="ExternalOutput").ap()

    RD = G * D   # 96 state rows
    RU = G * U   # 96 u rows
    RZ = G * O   # 48 z rows

    with tile.TileContext(nc) as tc:
        with (
            tc.tile_pool(name="const", bufs=1) as const,
            tc.tile_pool(name="stage", bufs=2) as stage,
            tc.tile_pool(name="fm", bufs=3) as fm,
            tc.tile_pool(name="ps_in", bufs=2, space="PSUM") as ps_in,
            tc.tile_pool(name="ps_z", bufs=2, space="PSUM") as ps_zp,
            tc.tile_pool(name="ps_st", bufs=2, space="PSUM") as ps_stp,
            tc.tile_pool(name="ps_out", bufs=2, space="PSUM") as ps_outp,
        ):
            id_t = const.tile([128, 128], f32)
            nc.sync.dma_start(id_t[:], ident[:])
            sm_t = const.tile([RD, T * RD], f32)
            nc.sync.dma_start(
                sm_t[:].rearrange("k (t m) -> k t m", t=T),
                statM.rearrange("t k m -> k t m"))
            sn_t = const.tile([RU, T * RD], f32)
            nc.sync.dma_start(
                sn_t[:].rearrange("k (t m) -> k t m", t=T),
                statN.rearrange("t k m -> k t m"))
            sk_t = const.tile([RZ, T * RD], f32)
            nc.sync.dma_start(
                sk_t[:].rearrange("k (t m) -> k t m", t=T),
                statK.rearrange("t k m -> k t m"))

            # initial state: load mean0 batch-major, transpose to [96, 256]
            m0 = stage.tile([128, 2 * RD], f32, tag="m0")
            nc.sync.dma_start(
                m0[:].rearrange("p (h f) -> p h f", h=2),
                mean0.rearrange("(h p q) i -> p h (q i)", h=2, p=128, q=G))
            ps0 = ps_in.tile([RD, COLS], f32, tag="ps_u")
            for h in range(2):
                nc.tensor.transpose(ps0[:, h * 128:(h + 1) * 128],
                                    m0[:, h * RD:(h + 1) * RD], id_t[:])
            state = fm.tile([RD, COLS], f32, tag="state")
            nc.scalar.copy(state[:], ps0[:])

            for grp in range(T // KT):
                u_st = stage.tile([128, KT * 2 * RD], f32, tag="u_st")
                nc.sync.dma_start(
                    u_st[:].rearrange("p (t h f) -> p t h f", t=KT, h=2),
                    useq[grp * KT:(grp + 1) * KT].rearrange(
                        "t (h p q) u -> p t h (q u)", h=2, p=128, q=G))
                z_st = stage.tile([128, KT * 2 * RZ], f32, tag="z_st")
                nc.sync.dma_start(
                    z_st[:].rearrange("p (t h f) -> p t h f", t=KT, h=2),
                    meas[grp * KT:(grp + 1) * KT].rearrange(
                        "t (h p q) o -> p t h (q o)", h=2, p=128, q=G))
                o_st = stage.tile([128, KT * 2 * RD], f32, tag="o_st")

                for tl in range(KT):
                    t = grp * KT + tl
                    # --- transpose u_t, z_t to feature-major ---
                    ps_u = ps_in.tile([RD, COLS], f32, tag="ps_u")
                    for h in range(2):
                        nc.tensor.transpose(
                            ps_u[:, h * 128:(h + 1) * 128],
                            u_st[:, (tl * 2 + h) * RD:(tl * 2 + h + 1) * RD],
                            id_t[:])
                    uT = fm.tile([RD, COLS], f32, tag="uT")
                    nc.scalar.copy(uT[:], ps_u[:])

                    ps_z = ps_zp.tile([RZ, COLS], f32, tag="ps_z")
                    for h in range(2):
                        nc.tensor.transpose(
                            ps_z[:, h * 128:(h + 1) * 128],
                            z_st[:, (tl * 2 + h) * RZ:(tl * 2 + h + 1) * RZ],
                            id_t[:])
                    zT = fm.tile([RZ, COLS], f32, tag="zT")
                    nc.vector.tensor_copy(zT[:], ps_z[:])

                    # --- state update: 3 accumulating matmuls ---
                    ps_s = ps_stp.tile([RD, COLS], f32, tag="ps_s")
                    nc.tensor.matmul(ps_s[:], sm_t[:, t * RD:(t + 1) * RD],
                                     state[:], start=True, stop=False)
                    nc.tensor.matmul(ps_s[:], sn_t[:, t * RD:(t + 1) * RD],
                                     uT[:], start=False, stop=False)
                    nc.tensor.matmul(ps_s[:], sk_t[:, t * RD:(t + 1) * RD],
                                     zT[:], start=False, stop=True)
                    state = fm.tile([RD, COLS], f32, tag="state")
                    nc.scalar.copy(state[:], ps_s[:])

                    # --- transpose new mean back to batch-major ---
                    ps_o = ps_outp.tile([128, 2 * RD], f32, tag="ps_o")
                    for h in range(2):
                        nc.tensor.transpose(
                            ps_o[:, h * RD:(h + 1) * RD],
                            state[:, h * 128:(h + 1) * 128],
                            id_t[:RD, :RD])
                    nc.vector.tensor_copy(
                        o_st[:, tl * 2 * RD:(tl + 1) * 2 * RD], ps_o[:])

                nc.sync.dma_start(
                    out[grp * KT:(grp + 1) * KT].rearrange(
                        "t (h p q) i -> p t h (q i)", h=2, p=128, q=G),
                    o_st[:].rearrange("p (t h f) -> p t h f", t=KT, h=2))

    nc.compile()
    _CACHE["nc"] = nc
    return nc


def _run_device(meas_np, useq_np, mean0_np, SM, SN, SK, trace=False):
    global LAST_RESULTS
    from concourse import bass_utils

    nc = _build_program()
    ident = np.eye(128, dtype=np.float32)
    in_maps = []
    for m in range(NCORES):
        sl = slice(m * BS, (m + 1) * BS)
        in_maps.append({
            "meas": np.ascontiguousarray(meas_np[:, sl]),
            "useq": np.ascontiguousarray(useq_np[:, sl]),
            "mean0": np.ascontiguousarray(mean0_np[sl]),
            "statM": SM, "statN": SN, "statK": SK, "ident": ident,
        })
    res = bass_utils.run_bass_kernel_spmd(
        nc, in_maps, core_ids=list(range(NCORES)), trace=trace)
    LAST_RESULTS = res
    return np.concatenate([res.results[m]["out"] for m in range(NCORES)], axis=1)


def _numpy_fallback(measurements, inputs_seq, mean0, cov0, A, Bm, Q_tril, C, R_tril):
    """General (per-batch covariance) EKF in vectorized numpy. Correctness
    fallback only; used when cov0 is not batch-uniform."""
    f = np.float32
    A = np.asarray(A, f); Bm = np.asarray(Bm, f); C = np.asarray(C, f)
    Qc = (np.asarray(Q_tril, f) @ np.asarray(Q_tril, f).T).astype(f)
    Rc = (np.asarray(R_tril, f) @ np.asarray(R_tril, f).T).astype(f)
    mean = np.asarray(mean0, f).copy()
    cov = np.asarray(cov0, f).copy()
    I = np.eye(D, dtype=f)
    outs = np.empty((T, mean.shape[0], D), f)
    for t in range(T):
        z = np.asarray(measurements[t], f)
        u = np.asarray(inputs_seq[t], f)
        pm = mean @ A.T + u @ Bm.T
        pc = np.einsum('ij,bjk,lk->bil', A, cov, A) + Qc
        innov = z - pm @ C.T
        S = np.einsum('ij,bjk,lk->bil', C, pc, C) + Rc
        PCt = np.einsum('bij,kj->bik', pc, C)
        K = PCt @ np.linalg.inv(S)
        mean = pm + np.einsum('bij,bj->bi', K, innov)
        cov = (I - np.einsum('bij,jk->bik', K, C)) @ pc
        outs[t] = mean
    return outs


def kernel(measurements, inputs_seq, mean0, cov0, A, Bm, Q_tril, C, R_tril):
    measurements = np.asarray(measurements)
    inputs_seq = np.asarray(inputs_seq)
    mean0 = np.asarray(mean0)
    cov0 = np.asarray(cov0)

    if np.ptp(cov0, axis=0).max() != 0.0:
        return _numpy_fallback(measurements, inputs_seq, mean0, cov0,
                               A, Bm, Q_tril, C, R_tril)

    Ms, Ns, Ks = _host_coeffs(cov0[0], A, Bm, Q_tril, C, R_tril)
    SM, SN, SK = _stationaries(Ms, Ns, Ks)
    return _run_device(measurements.astype(np.float32),
                       inputs_seq.astype(np.float32),
                       mean0.astype(np.float32), SM, SN, SK,
                       trace=False)


# revision 16
# speedup vs baseline: 1.2452x; 1.2452x over previous
"""Extended Kalman Filter kernel for 8 Trainium2 NeuronCores.

Math: the EKF covariance recursion (P -> A P A^T + Q; S = C P C^T + R;
K = P C^T S^-1; P -> (I-KC)P) does not depend on the data, only on cov0.
When cov0 is identical across the batch (it is: broadcast 0.1*I), the
per-timestep Kalman gains K_t are batch-independent and can be
precomputed on the host. The device-side work collapses to a linear
time-varying recursion on the mean only:

    mean_{t+1} = M_t @ mean_t + N_t @ u_t + K_t @ z_t
    M_t = (I - K_t C) A,  N_t = (I - K_t C) Bm

Device mapping (pure data-parallel over batch, 4096 batch/core):
  * batch n in [0,4096) is split as n = h*2048 + p*16 + q with h in {0,1},
    p in [0,128) (SBUF partition of the batch-major staging tiles),
    q in [0,16) (position within a partition's contiguous 16-batch run).
  * On-chip state layout is "feature-major blocks": mean tile
    [96 = (q,i), 256 = (h,p)] so the recursion is a matmul with a
    block-diagonal stationary kron(I_16, M_t^T) of shape [96, 96].
  * u_t / z_t arrive batch-major (contiguous DMA), are transposed
    on the TensorEngine ([128, 96] / [128, 48] tiles -> PSUM), copied to
    SBUF, and injected with block-diagonal stationaries.
  * The updated mean (= the output for step t) is transposed back to
    batch-major on the TensorEngine and stored contiguously.
"""

import numpy as np

T, BFULL, D, O, U = 64, 32768, 6, 3, 6
NCORES = 8
BS = BFULL // NCORES      # 4096 batch per core
G = 16                    # batches per 6-row feature block (96 = G*D rows)
COLS = 256                # state columns = 2 halves * 128 partitions
KT = 8                    # timesteps per DMA staging group

_CACHE = {}
LAST_RESULTS = None       # BassKernelResults of the most recent device run


def _host_coeffs(cov0_row, A, Bm, Q_tril, C, R_tril):
    """Run the (batch-independent) covariance recursion on the host in
    float64; return per-step float32 coefficient matrices M_t, N_t, K_t."""
    A = np.asarray(A, np.float64)
    Bm = np.asarray(Bm, np.float64)
    Qt = np.asarray(Q_tril, np.float64)
    C = np.asarray(C, np.float64)
    Rt = np.asarray(R_tril, np.float64)
    Qc = Qt @ Qt.T
    Rc = Rt @ Rt.T
    P = np.asarray(cov0_row, np.float64)
    I = np.eye(D)
    Ms = np.empty((T, D, D), np.float32)
    Ns = np.empty((T, D, U), np.float32)
    Ks = np.empty((T, D, O), np.float32)
    for t in range(T):
        Pp = A @ P @ A.T + Qc
        S = C @ Pp @ C.T + Rc
        K = Pp @ C.T @ np.linalg.inv(S)
        IKC = I - K @ C
        Ms[t] = IKC @ A
        Ns[t] = IKC @ Bm
        Ks[t] = K
        P = IKC @ Pp
    return Ms, Ns, Ks


def _stationaries(Ms, Ns, Ks):
    """Block-diagonal lhsT stationaries, packed for the two combined
    matmuls. matmul computes lhsT.T @ rhs, so each diagonal block is the
    transpose of the coefficient matrix.

    rhs1 (combo1) rows = [mean (96) ; zT rows 0:32], lhsT1 [128, 96]
    rhs2 (combo2) rows = [zT rows 32:48 ; uT (96)], lhsT2 [112, 96]
    """
    SM = np.zeros((T, G * D, G * D), np.float32)
    SN = np.zeros((T, G * U, G * D), np.float32)
    SK = np.zeros((T, G * O, G * D), np.float32)
    for g in range(G):
        SM[:, g * D:(g + 1) * D, g * D:(g + 1) * D] = np.transpose(Ms, (0, 2, 1))
        SN[:, g * U:(g + 1) * U, g * D:(g + 1) * D] = np.transpose(Ns, (0, 2, 1))
        SK[:, g * O:(g + 1) * O, g * D:(g + 1) * D] = np.transpose(Ks, (0, 2, 1))
    S1 = np.concatenate([SM, SK[:, 0:32, :]], axis=1)          # [T, 128, 96]
    S2 = np.concatenate([SN, SK[:, 32:48, :]], axis=1)         # [T, 112, 96]
    return np.ascontiguousarray(S1), np.ascontiguousarray(S2)


def _build_program():
    """Build (once) the Bass/Tile program shared by all 8 cores."""
    if "nc" in _CACHE:
        return _CACHE["nc"]

    import concourse.bacc as bacc
    import concourse.tile as tile
    from concourse import mybir

    f32 = mybir.dt.float32
    nc = bacc.Bacc("TRN2", target_bir_lowering=False, debug=False,
                   num_devices=NCORES)

    meas = nc.dram_tensor("meas", [T, BS, O], f32, kind="ExternalInput").ap()
    useq = nc.dram_tensor("useq", [T, BS, U], f32, kind="ExternalInput").ap()
    mean0 = nc.dram_tensor("mean0", [BS, D], f32, kind="ExternalInput").ap()
    stat1 = nc.dram_tensor("stat1", [T, 128, G * D], f32, kind="ExternalInput").ap()
    stat2 = nc.dram_tensor("stat2", [T, 112, G * D], f32, kind="ExternalInput").ap()
    ident = nc.dram_tensor("ident", [128, 128], f32, kind="ExternalInput").ap()
    out = nc.dram_tensor("out", [T, BS, D], f32, kind="ExternalOutput").ap()

    RD = G * D   # 96 state rows
    RZ = G * O   # 48 z rows
    NG = T // KT

    with tile.TileContext(nc) as tc:
        with (
            tc.tile_pool(name="const", bufs=1) as const,
            tc.tile_pool(name="stage", bufs=2) as stage,
            tc.tile_pool(name="fm", bufs=16) as fm,
            tc.tile_pool(name="ps_u", bufs=3, space="PSUM") as ps_up,
            tc.tile_pool(name="ps_z", bufs=2, space="PSUM") as ps_zp,
            tc.tile_pool(name="ps_s", bufs=1, space="PSUM") as ps_sp,
            tc.tile_pool(name="ps_o", bufs=2, space="PSUM") as ps_op,
        ):
            id_t = const.tile([128, 128], f32)
            nc.sync.dma_start(id_t[:], ident[:])
            s1_t = const.tile([128, T * RD], f32)
            s2_t = const.tile([112, T * RD], f32)
            for g in range(NG):
                sl = slice(g * KT, (g + 1) * KT)
                fs = slice(g * KT * RD, (g + 1) * KT * RD)
                nc.sync.dma_start(
                    s1_t[:, fs].rearrange("k (t m) -> k t m", t=KT),
                    stat1[sl].rearrange("t k m -> k t m"))
                nc.sync.dma_start(
                    s2_t[:, fs].rearrange("k (t m) -> k t m", t=KT),
                    stat2[sl].rearrange("t k m -> k t m"))

            # combo1(t) = [mean_t (96 rows) ; zT_t rows 0:32]  -> lhsT stat1
            # combo2(t) = [uT_t (96 rows) ; zT_t rows 32:48]   -> lhsT stat2
            combo1 = [fm.tile([128, COLS], f32, tag="c1", name=f"c1_{i}") for i in range(KT)]
            combo2 = [fm.tile([112, COLS], f32, tag="c2", name=f"c2_{i}") for i in range(KT)]

            # initial state: load mean0 batch-major, transpose into combo1[0]
            m0 = stage.tile([128, 2 * RD], f32, tag="m0")
            nc.sync.dma_start(
                m0[:].rearrange("p (h f) -> p h f", h=2),
                mean0.rearrange("(h p q) i -> p h (q i)", h=2, p=128, q=G))
            ps0 = ps_up.tile([RD, COLS], f32, tag="ps_u")
            for h in range(2):
                nc.tensor.transpose(ps0[:, h * 128:(h + 1) * 128],
                                    m0[:, h * RD:(h + 1) * RD], id_t[:])
            nc.scalar.copy(combo1[0][0:RD, :], ps0[:])

            u_sts, z_sts, o_sts = {}, {}, {}

            def load_group(g):
                u_st = stage.tile([128, KT * 2 * RD], f32, tag="u_st")
                nc.sync.dma_start(
                    u_st[:].rearrange("p (t h f) -> p t h f", t=KT, h=2),
                    useq[g * KT:(g + 1) * KT].rearrange(
                        "t (h p q) u -> p t h (q u)", h=2, p=128, q=G))
                z_st = stage.tile([128, KT * 2 * RZ], f32, tag="z_st")
                nc.sync.dma_start(
                    z_st[:].rearrange("p (t h f) -> p t h f", t=KT, h=2),
                    meas[g * KT:(g + 1) * KT].rearrange(
                        "t (h p q) o -> p t h (q o)", h=2, p=128, q=G))
                u_sts[g], z_sts[g] = u_st, z_st

            def transpose_step(t, c1, c2):
                """PE transposes + copies filling combo tiles for step t."""
                g, tl = t // KT, t % KT
                ps_u = ps_up.tile([RD, COLS], f32, tag="ps_u")
                for h in range(2):
                    nc.tensor.transpose(
                        ps_u[:, h * 128:(h + 1) * 128],
                        u_sts[g][:, (tl * 2 + h) * RD:(tl * 2 + h + 1) * RD],
                        id_t[:])
                ps_z = ps_zp.tile([RZ, COLS], f32, tag="ps_z")
                for h in range(2):
                    nc.tensor.transpose(
                        ps_z[:, h * 128:(h + 1) * 128],
                        z_sts[g][:, (tl * 2 + h) * RZ:(tl * 2 + h + 1) * RZ],
                        id_t[:])
                nc.scalar.copy(c2[0:RD, :], ps_u[:])
                nc.vector.tensor_copy(c1[RD:128, :], ps_z[0:32, :])
                nc.vector.tensor_copy(c2[RD:112, :], ps_z[32:48, :])

            def out_transpose(t, c1_next):
                """Transpose mean_{t+1} (= output t) to batch-major."""
                g, tl = t // KT, t % KT
                ps_o = ps_op.tile([128, 2 * RD], f32, tag="ps_o")
                for h in range(2):
                    nc.tensor.transpose(
                        ps_o[:, h * RD:(h + 1) * RD],
                        c1_next[0:RD, h * 128:(h + 1) * 128],
                        id_t[:RD, :RD])
                nc.vector.tensor_copy(
                    o_sts[g][:, tl * 2 * RD:(tl + 1) * 2 * RD], ps_o[:])

            # prologue: group 0 (and its transposes), issue group-1 loads
            load_group(0)
            for tl in range(KT):
                transpose_step(tl, combo1[tl], combo2[tl])

            for g in range(NG):
                o_sts[g] = stage.tile([128, KT * 2 * RD], f32, tag="o_st",
                                      name=f"o_st_{g}")
                if g + 1 < NG:
                    load_group(g + 1)
                combo1_next = [fm.tile([128, COLS], f32, tag="c1",
                                        name=f"c1_{g + 1}_{i}") for i in range(KT)]
                combo2_next = [fm.tile([112, COLS], f32, tag="c2",
                                        name=f"c2_{g + 1}_{i}") for i in range(KT)]
                for tl in range(KT):
                    t = g * KT + tl
                    # chain: state(t+1) = stat1_t.T @ combo1(t) + stat2_t.T @ combo2(t)
                    ps_s = ps_sp.tile([RD, COLS], f32, tag="ps_s")
                    nc.tensor.matmul(ps_s[:], s1_t[:, t * RD:(t + 1) * RD],
                                     combo1[tl][:], start=True, stop=False)
                    nc.tensor.matmul(ps_s[:], s2_t[:, t * RD:(t + 1) * RD],
                                     combo2[tl][:], start=False, stop=True)
                    c1n = combo1_next[0] if tl == KT - 1 else combo1[tl + 1]
                    nc.scalar.copy(c1n[0:RD, 0:128], ps_s[:, 0:128])
                    nc.vector.tensor_copy(c1n[0:RD, 128:COLS], ps_s[:, 128:COLS])
                    # fill PE pipeline while the state copy is in flight:
                    if t > 0:
                        # mean_t (= output t-1) lives in combo1[tl][0:96]
                        out_transpose(t - 1, combo1[tl])
                    if g + 1 < NG:
                        transpose_step((g + 1) * KT + tl,
                                       combo1_next[tl], combo2_next[tl])
                    if tl == 0 and g > 0:
                        nc.sync.dma_start(
                            out[(g - 1) * KT:g * KT].rearrange(
                                "t (h p q) i -> p t h (q i)", h=2, p=128, q=G),
                            o_sts[g - 1][:].rearrange(
                                "p (t h f) -> p t h f", t=KT, h=2))
                combo1, combo2 = combo1_next, combo2_next

            # epilogue: final output transpose + last group store
            out_transpose(T - 1, combo1[0])
            nc.sync.dma_start(
                out[(NG - 1) * KT:].rearrange(
                    "t (h p q) i -> p t h (q i)", h=2, p=128, q=G),
                o_sts[NG - 1][:].rearrange("p (t h f) -> p t h f", t=KT, h=2))

    nc.compile()
    _CACHE["nc"] = nc
    return nc


def _run_device(meas_np, useq_np, mean0_np, S1, S2, trace=False):
    global LAST_RESULTS
    from concourse import bass_utils

    nc = _build_program()
    ident = np.eye(128, dtype=np.float32)
    in_maps = []
    for m in range(NCORES):
        sl = slice(m * BS, (m + 1) * BS)
        in_maps.append({
            "meas": np.ascontiguousarray(meas_np[:, sl]),
            "useq": np.ascontiguousarray(useq_np[:, sl]),
            "mean0": np.ascontiguousarray(mean0_np[sl]),
            "stat1": S1, "stat2": S2, "ident": ident,
        })
    res = bass_utils.run_bass_kernel_spmd(
        nc, in_maps, core_ids=list(range(NCORES)), trace=trace)
    LAST_RESULTS = res
    return np.concatenate([res.results[m]["out"] for m in range(NCORES)], axis=1)


def _numpy_fallback(measurements, inputs_seq, mean0, cov0, A, Bm, Q_tril, C, R_tril):
    """General (per-batch covariance) EKF in vectorized numpy. Correctness
    fallback only; used when cov0 is not batch-uniform."""
    f = np.float32
    A = np.asarray(A, f); Bm = np.asarray(Bm, f); C = np.asarray(C, f)
    Qc = (np.asarray(Q_tril, f) @ np.asarray(Q_tril, f).T).astype(f)
    Rc = (np.asarray(R_tril, f) @ np.asarray(R_tril, f).T).astype(f)
    mean = np.asarray(mean0, f).copy()
    cov = np.asarray(cov0, f).copy()
    I = np.eye(D, dtype=f)
    outs = np.empty((T, mean.shape[0], D), f)
    for t in range(T):
        z = np.asarray(measurements[t], f)
        u = np.asarray(inputs_seq[t], f)
        pm = mean @ A.T + u @ Bm.T
        pc = np.einsum('ij,bjk,lk->bil', A, cov, A) + Qc
        innov = z - pm @ C.T
        S = np.einsum('ij,bjk,lk->bil', C, pc, C) + Rc
        PCt = np.einsum('bij,kj->bik', pc, C)
        K = PCt @ np.linalg.inv(S)
        mean = pm + np.einsum('bij,bj->bi', K, innov)
        cov = (I - np.einsum('bij,jk->bik', K, C)) @ pc
        outs[t] = mean
    return outs


def kernel(measurements, inputs_seq, mean0, cov0, A, Bm, Q_tril, C, R_tril):
    measurements = np.asarray(measurements)
    inputs_seq = np.asarray(inputs_seq)
    mean0 = np.asarray(mean0)
    cov0 = np.asarray(cov0)

    if np.ptp(cov0, axis=0).max() != 0.0:
        return _numpy_fallback(measurements, inputs_seq, mean0, cov0,
                               A, Bm, Q_tril, C, R_tril)

    Ms, Ns, Ks = _host_coeffs(cov0[0], A, Bm, Q_tril, C, R_tril)
    S1, S2 = _stationaries(Ms, Ns, Ks)
    return _run_device(measurements.astype(np.float32),
                       inputs_seq.astype(np.float32),
                       mean0.astype(np.float32), S1, S2,
                       trace=False)
